# revision 1
# baseline (speedup 1.0000x reference)
"""Correlation1d (FlowNetC/DispNetC) Trainium2 Bass kernel.

out[b, i, h, w] = (1/C) * sum_c in1[b,c,h,w] * in2[b,c,h,w + d_i],
d_i = -20 + 2i, i in [0, 21), out-of-range -> 0.

Strategy (data-parallel over batch, one batch per NeuronCore):
  - Per (h): Gram matrix M_h = in1_h^T @ in2_h  ([w, w'] = sum_c ...) on the
    tensor engine in fp32 (two K=128 accumulating matmuls for C=256).
  - Evacuate PSUM -> SBUF with a strided DVE copy into layout [w, w', h]
    (h innermost) while applying the 1/C scale.
  - Band extraction ("shear"): 128 single-partition DMAs, one per w, each
    copying the 21 even-offset band elements M[w, w-20+2k, :] (h-contiguous
    runs) into out_pre[w, k, h].
  - PE-transpose out_pre [w, (k h)] -> [(k h), w] in 128-column blocks and
    write each block contiguously to the output (out[(i h), w] row-major).
"""
import sys
import time

sys.path.insert(0, '/opt/trn_rl_repo')

import numpy as np

B, C, H, W = 8, 256, 64, 128
MAX_DISP, STRIDE2 = 20, 2
ND = 2 * (MAX_DISP // STRIDE2) + 1   # 21 displacement channels
BAND = ND
N_CORES = 8
CHUNK = 8                            # h per streamed input chunk
SCALE = 1.0 / C

_cache = {}


def _build():
    import concourse.bass as bass
    import concourse.mybir as mybir
    import concourse.tile as tile
    from concourse import bacc
    from concourse.masks import make_identity

    F32 = mybir.dt.float32
    nc = bacc.Bacc('TRN2', target_bir_lowering=False, debug=False)
    in1 = nc.declare_dram_parameter("in1", [C, H, W], F32, isOutput=False)
    in2 = nc.declare_dram_parameter("in2", [C, H, W], F32, isOutput=False)
    out = nc.declare_dram_parameter("out", [ND, H, W], F32, isOutput=True)
    out_flat = out.rearrange("i h w -> (i h) w")

    with tile.TileContext(nc) as tc:
        with tc.tile_pool(name="const", bufs=1) as const_pool, \
             tc.tile_pool(name="ins", bufs=3) as ins_pool, \
             tc.tile_pool(name="msb", bufs=1) as msb_pool, \
             tc.tile_pool(name="opre", bufs=1) as opre_pool, \
             tc.tile_pool(name="tsb", bufs=2) as tsb_pool, \
             tc.tile_pool(name="psum_m", bufs=4, space="PSUM") as psum_m, \
             tc.tile_pool(name="psum_t", bufs=2, space="PSUM") as psum_t:

            ident = const_pool.tile([128, 128], F32)
            make_identity(nc, ident)

            M_sb = msb_pool.tile([128, W, H], F32)          # [w, w', h]
            out_pre = opre_pool.tile([128, BAND, H], F32)   # [w, k, h]
            nc.vector.memset(out_pre, 0.0)

            for h0 in range(0, H, CHUNK):
                t1, t2 = [], []
                for cb in range(2):
                    a = ins_pool.tile([128, CHUNK, W], F32, tag=f"in1c{cb}")
                    nc.sync.dma_start(out=a, in_=in1[cb * 128:(cb + 1) * 128, h0:h0 + CHUNK, :])
                    t1.append(a)
                    b = ins_pool.tile([128, CHUNK, W], F32, tag=f"in2c{cb}")
                    nc.sync.dma_start(out=b, in_=in2[cb * 128:(cb + 1) * 128, h0:h0 + CHUNK, :])
                    t2.append(b)
                for hh in range(CHUNK):
                    h = h0 + hh
                    pm = psum_m.tile([128, W], F32)
                    nc.tensor.matmul(pm, t1[0][:, hh, :], t2[0][:, hh, :], start=True, stop=False)
                    nc.tensor.matmul(pm, t1[1][:, hh, :], t2[1][:, hh, :], start=False, stop=True)
                    nc.vector.tensor_scalar_mul(M_sb[:, :, h], pm, SCALE)

            # band extraction: out_pre[w, k, :] = M_sb[w, w-20+2k, :]
            for w in range(128):
                k0 = max(0, (21 - w) // 2)
                k1 = min(BAND, (149 - w) // 2)
                src = bass.AP(tensor=M_sb.tensor,
                              offset=w * (W * H) + (w - 20 + 2 * k0) * H,
                              ap=[[W * H, 1], [2 * H, k1 - k0], [1, H]])
                dst = bass.AP(tensor=out_pre.tensor,
                              offset=w * (BAND * H) + k0 * H,
                              ap=[[BAND * H, 1], [H, k1 - k0], [1, H]])
                eng = nc.sync if (w % 2 == 0) else nc.scalar
                eng.dma_start(out=dst, in_=src)

            # transpose [w, (k h)] -> [(k h), w] in 128-col blocks, write out
            KHTOT = BAND * H  # 1344
            opre_flat = out_pre.rearrange("w k h -> w (k h)")
            for t0 in range(0, KHTOT, 128):
                size = min(128, KHTOT - t0)
                pt = psum_t.tile([size, 128], F32)
                nc.tensor.transpose(pt, opre_flat[:, t0:t0 + size], ident)
                ts = tsb_pool.tile([size, 128], F32, tag="tout")
                nc.vector.tensor_copy(ts, pt)
                nc.sync.dma_start(out=out_flat[t0:t0 + size, :], in_=ts)

    nc.finalize()
    return nc


def _get_nc():
    if "nc" not in _cache:
        _cache["nc"] = _build()
    return _cache["nc"]


def kernel(input1: np.ndarray, input2: np.ndarray) -> np.ndarray:
    from concourse.bass_utils import run_bass_kernel_spmd

    input1 = np.ascontiguousarray(input1, dtype=np.float32)
    input2 = np.ascontiguousarray(input2, dtype=np.float32)
    assert input1.shape == (B, C, H, W) and input2.shape == (B, C, H, W)

    nc = _get_nc()
    in_maps = [{"in1": input1[b], "in2": input2[b]} for b in range(N_CORES)]
    results = run_bass_kernel_spmd(nc, in_maps, list(range(N_CORES))).results
    return np.stack([results[b]["out"] for b in range(N_CORES)], axis=0)


if __name__ == "__main__":
    rng = np.random.default_rng(0)
    i1 = rng.standard_normal((B, C, H, W)).astype(np.float32)
    i2 = rng.standard_normal((B, C, H, W)).astype(np.float32)
    t0 = time.time()
    o = kernel(i1, i2)
    print("kernel done in", time.time() - t0, "s; out shape", o.shape)
